# revision 2
# baseline (speedup 1.0000x reference)
"""nn_DiTBlock Trainium2 kernel: 8-core sharded AdaLN-Zero DiT block.

Sharding: 8 cores = 4 batch elements x 2 query-halves. Each core receives
its batch element's tokens rolled so its own 1024 query tokens come first,
computes K/V over all 2048 tokens, and attention/FFN/output for its own
1024 query rows. No collectives; per-core outputs are disjoint row blocks.
All matmuls run in float32r (fp32 data, FP22 multiply, fp32 accumulate).
"""
import sys
sys.path.insert(0, "/opt/trn_rl_repo")

import numpy as np

from contextlib import ExitStack

import concourse.bass as bass
import concourse.tile as tile
from concourse import mybir
from concourse.masks import make_identity

F32 = mybir.dt.float32
F32R = mybir.dt.float32r
AF = mybir.ActivationFunctionType
ALU = mybir.AluOpType

P = 128
EPS = 1e-5


def ap2(handle, offset, ap):
    return bass.AP(tensor=handle, offset=offset, ap=[list(p) for p in ap])


def build_dit(nc, D=1024, NH=16, DFF=4096, NT=2048, NQ=1024, TOK_BLK=512,
              GELU_FUNC=AF.Gelu):
    HD = 64
    assert NH * HD == D
    DC = D // P            # feature chunks of d_model
    KT = NT // P           # kv token tiles
    NSL = min(512, NQ)     # matmul N slice over queries
    NBLK = NT // TOK_BLK   # token blocks for streaming
    TT = TOK_BLK // P      # token tiles per block
    FC = DFF // P          # d_ff chunks
    FCB = 4                # d_ff chunks per FFN block
    FB = FC // FCB
    QS = 4                 # token quarters for FFN
    NQQ = NQ // QS
    HC = 65                # head cols in V_aug (64 data + 1 ones)

    xb = nc.dram_tensor("xb", [NT, D], F32, kind="ExternalInput")
    cb = nc.dram_tensor("cb", [1, D], F32, kind="ExternalInput")
    W_ada = nc.dram_tensor("W_ada", [D, 6 * D], F32, kind="ExternalInput")
    b_ada = nc.dram_tensor("b_ada", [1, 6 * D], F32, kind="ExternalInput")
    Wq = nc.dram_tensor("Wq", [D, D], F32, kind="ExternalInput")
    bq = nc.dram_tensor("bq", [1, D], F32, kind="ExternalInput")
    Wk = nc.dram_tensor("Wk", [D, D], F32, kind="ExternalInput")
    bk = nc.dram_tensor("bk", [1, D], F32, kind="ExternalInput")
    Wv = nc.dram_tensor("Wv", [D, D], F32, kind="ExternalInput")
    bv = nc.dram_tensor("bv", [1, D], F32, kind="ExternalInput")
    Wo = nc.dram_tensor("Wo", [D, D], F32, kind="ExternalInput")
    bo = nc.dram_tensor("bo", [1, D], F32, kind="ExternalInput")
    W1 = nc.dram_tensor("W1", [D, DFF], F32, kind="ExternalInput")
    b1 = nc.dram_tensor("b1", [1, DFF], F32, kind="ExternalInput")
    W2 = nc.dram_tensor("W2", [DFF, D], F32, kind="ExternalInput")
    b2 = nc.dram_tensor("b2", [1, D], F32, kind="ExternalInput")
    out = nc.dram_tensor("out", [NQ, D], F32, kind="ExternalOutput")

    with tile.TileContext(nc) as tc, ExitStack() as ctx:
        dram = ctx.enter_context(tc.tile_pool(name="dram", bufs=1, space="DRAM"))
        ada_dram = dram.tile([1, 6 * D], F32)
        q_drams = [dram.tile([P, NQ], F32R, name=f"q_dram{c}")
                   for c in range(DC)]
        k_drams = [dram.tile([P, NT], F32R, name=f"k_dram{c}")
                   for c in range(DC)]
        adh = ada_dram.tensor

        consts = ctx.enter_context(tc.tile_pool(name="consts", bufs=1))
        ident = consts.tile([P, P], F32)
        make_identity(nc, ident[:])
        # packed small constants: [eps, one, bk(8), bq(8), bo(8), b1(32),
        # s1(8), sh1(8), s2(8), sh2(8)] -> one padded tile
        pack = consts.tile([P, 96], F32)
        eps_t = pack[:, 0:1]
        nc.vector.memset(eps_t, EPS)
        one_col = pack[:, 1:2]
        nc.vector.memset(one_col, 1.0)
        bk_pp = pack[:, 2:2 + DC]
        nc.sync.dma_start(bk_pp, ap2(bk, 0, [[1, P], [P, DC]]))
        bq_pp = pack[:, 10:10 + DC]
        nc.sync.dma_start(bq_pp, ap2(bq, 0, [[1, P], [P, DC]]))
        bo_pp = pack[:, 18:18 + DC]
        nc.sync.dma_start(bo_pp, ap2(bo, 0, [[1, P], [P, DC]]))
        b1_pp = pack[:, 26:26 + FC]
        nc.sync.dma_start(b1_pp, ap2(b1, 0, [[1, P], [P, FC]]))

        _pp_next = [58 + FC - 32]  # pack cols from 58 when FC=32

        def load_pp(pool, name, off, plus1=False):
            """ada slice as per-partition chunked [P, DC] into pack cols."""
            c0 = _pp_next[0]
            assert c0 + DC <= 96
            t = pack[:, c0:c0 + DC]
            _pp_next[0] = c0 + DC
            nc.sync.dma_start(t, ap2(adh, off, [[1, P], [P, DC]]))
            if plus1:
                nc.vector.tensor_scalar(t, t, scalar1=one_col,
                                        scalar2=None, op0=ALU.add)
            return t

        def load_bc(pool, name, dram_handle, off):
            t = pool.tile([P, D], F32, name=name)
            nc.sync.dma_start(t[:], ap2(dram_handle, off, [[0, P], [1, D]]))
            return t

        def ln_normalize(pool, xt):
            """token-major LN (no affine): (x - mean) * rsqrt(var + eps).
            All per-token scalars live in one packed tile (pad economy)."""
            lp = pool.tile([P, 16], F32, tag="lnp")
            stats = lp[:, 0:12].rearrange("p (s f) -> p s f", f=6)
            xv = xt[:].rearrange("p (s f) -> p s f", f=512)
            for s in range(D // 512):
                nc.vector.bn_stats(stats[:, s, :], xv[:, s, :])
            mv = lp[:, 12:14]
            nc.vector.bn_aggr(mv, lp[:, 0:12].rearrange(
                "p (s f) -> p s f", f=6)[:, :D // 512, :])
            sd = lp[:, 14:15]
            nc.scalar.activation(sd, mv[:, 1:2], AF.Sqrt, bias=eps_t)
            rstd = lp[:, 15:16]
            nc.vector.reciprocal(rstd, sd)
            xn = pool.tile([P, D], F32, tag="xn")
            nc.vector.tensor_scalar(xn[:], xt[:], scalar1=mv[:, 0:1],
                                    scalar2=rstd,
                                    op0=ALU.subtract, op1=ALU.mult)
            return xn

        # ============ Phase A: ada = cb @ W_ada + b_ada -> ada_dram
        with tc.tile_pool(name="ada_w", bufs=3) as awp, \
             tc.tile_pool(name="ada_sb", bufs=3) as asb, \
             tc.tile_pool(name="ada_ps", bufs=2, space="PSUM") as aps:
            cT = asb.tile([P, DC], F32R)
            nc.sync.dma_start(cT[:], ap2(cb, 0, [[1, P], [P, DC]]).bitcast(F32R))
            for j in range(6 * D // 512):
                ps = aps.tile([1, 512], F32, tag="ps")
                for kc in range(DC):
                    wt = awp.tile([P, 512], F32R, tag="w")
                    enga = nc.scalar if kc % 2 == 0 else nc.sync
                    enga.dma_start(
                        wt[:], ap2(W_ada, kc * P * 6 * D + j * 512,
                                   [[6 * D, P], [1, 512]]).bitcast(F32R))
                    nc.tensor.matmul(ps[:], cT[:, kc:kc + 1], wt[:],
                                     start=(kc == 0), stop=(kc == DC - 1))
                bt = asb.tile([1, 512], F32, tag="b")
                nc.sync.dma_start(bt[:], ap2(b_ada, j * 512, [[512, 1], [1, 512]]))
                st = asb.tile([1, 512], F32, tag="s")
                nc.vector.tensor_tensor(st[:], ps[:], bt[:], op=ALU.add)
                nc.sync.dma_start(ap2(adh, j * 512, [[512, 1], [1, 512]]), st[:])

        ores = ctx.enter_context(tc.tile_pool(name="ores", bufs=1, side="right"))
        oT = ores.tile([P, DC, NQ], F32R, tag="big3")
        with tc.tile_pool(name="vres", bufs=1) as vres:
            with tc.tile_pool(name="hres", bufs=1) as hres_pool:
                hres = hres_pool.tile([P, DC, NT], F32R)
                # ======== Phase C-LN: LN1 + transpose + fused modulation
                with tc.tile_pool(name="mod1", bufs=1) as mod1, \
                     tc.tile_pool(name="ln1", bufs=3) as lnp, \
                     tc.tile_pool(name="tps", bufs=4, space="PSUM") as tps:
                    s1_pp = load_pp(mod1, "s1_pp", 1 * D, plus1=True)
                    sh1_pp = load_pp(mod1, "sh1_pp", 0 * D)
                    for t in range(KT):
                        xt = lnp.tile([P, D], F32, tag="x")
                        nc.sync.dma_start(xt[:], ap2(xb, t * P * D,
                                                     [[D, P], [1, D]]))
                        xn = ln_normalize(lnp, xt)
                        for dc in range(DC):
                            pt = tps.tile([P, P], F32, tag="t")
                            nc.tensor.transpose(pt[:], xn[:, dc * P:(dc + 1) * P],
                                                ident[:])
                            if dc % 2 == 0:
                                nc.vector.tensor_scalar(
                                    hres[:, dc, t * P:(t + 1) * P], pt[:],
                                    scalar1=s1_pp[:, dc:dc + 1],
                                    scalar2=sh1_pp[:, dc:dc + 1],
                                    op0=ALU.mult, op1=ALU.add)
                            else:
                                nc.scalar.activation(
                                    hres[:, dc, t * P:(t + 1) * P], pt[:],
                                    AF.Identity,
                                    scale=s1_pp[:, dc:dc + 1],
                                    bias=sh1_pp[:, dc:dc + 1])

                # ======== Phase C-KQ: K^T, Q^T -> dram (weights read once,
                # mc-outer so early head-pairs' inputs finish first)
                with tc.tile_pool(name="wkq", bufs=24) as wkq, \
                     tc.tile_pool(name="qev", bufs=3) as qev, \
                     tc.tile_pool(name="kqps", bufs=2, space="PSUM") as kqps:
                    for mc in range(DC):
                        for W_, b_pp, is_q in ((Wk, bk_pp, False),
                                               (Wq, bq_pp, True)):
                            ncols = NQ if is_q else NT
                            wsl = min(512, ncols)
                            wcol = []
                            for kc in range(DC):
                                wt = wkq.tile([P, P], F32R, tag="w")
                                eng = nc.scalar if kc % 2 == 0 else nc.sync
                                eng.dma_start(
                                    wt[:], ap2(W_, kc * P * D + mc * P,
                                               [[D, P], [1, P]]).bitcast(F32R))
                                wcol.append(wt)
                            for ns in range(ncols // wsl):
                                ps = kqps.tile([P, 512], F32, tag="ps")
                                for kc in range(DC):
                                    nc.tensor.matmul(
                                        ps[:, :wsl], wcol[kc][:],
                                        hres[:, kc, ns * wsl:(ns + 1) * wsl],
                                        start=(kc == 0), stop=(kc == DC - 1))
                                qe = qev.tile([P, 512], F32R, tag="qe")
                                nc.vector.tensor_scalar(
                                    qe[:, :wsl], ps[:, :wsl],
                                    scalar1=b_pp[:, mc:mc + 1],
                                    scalar2=None, op0=ALU.add)
                                dst = q_drams[mc] if is_q else k_drams[mc]
                                dcols = NQ if is_q else NT
                                nc.gpsimd.dma_start(
                                    ap2(dst.tensor, ns * wsl,
                                        [[dcols, P], [1, wsl]]), qe[:, :wsl])

                V_aug = vres.tile([P, KT, NH * HC], F32R)

                def compute_v_half(wvp, vps, bv_bc, nh):
                    wvt = []
                    for kc in range(DC):
                        wt = wvp.tile([P, 512], F32R, tag="wv")
                        eng = nc.scalar if kc % 2 == 0 else nc.sync
                        eng.dma_start(
                            wt[:], ap2(Wv, kc * P * D + nh * 512,
                                       [[D, P], [1, 512]]).bitcast(F32R))
                        wvt.append(wt)
                    for t in range(KT):
                        ps = vps.tile([P, 512], F32, tag="ps")
                        for kc in range(DC):
                            nc.tensor.matmul(
                                ps[:], hres[:, kc, t * P:(t + 1) * P],
                                wvt[kc][:],
                                start=(kc == 0), stop=(kc == DC - 1))
                        dst = V_aug[:, t,
                                    nh * 8 * HC:(nh + 1) * 8 * HC].rearrange(
                            "p (h c) -> p h c", c=HC)[:, :, 0:HD]
                        nc.vector.tensor_tensor(
                            dst, ps[:].rearrange("p (h c) -> p h c", c=HD),
                            bv_bc[:, nh * 512:(nh + 1) * 512].rearrange(
                                "p (h c) -> p h c", c=HD),
                            op=ALU.add)

                # ======== Phase C-V: both halves of V
                with tc.tile_pool(name="mod1v", bufs=1) as mod1v, \
                     tc.tile_pool(name="wv", bufs=9) as wvp, \
                     tc.tile_pool(name="vps", bufs=2, space="PSUM") as vps:
                    nc.scalar.copy(
                        V_aug[:].rearrange("p t (h c) -> p t h c",
                                           c=HC)[:, :, :, HD:HD + 1],
                        one_col.to_broadcast((P, KT, NH, 1)))
                    bv_bc = load_bc(mod1v, "bv_bc", bv, 0)
                    for nh in range(D // 512):
                        compute_v_half(wvp, vps, bv_bc, nh)

            # ============ Phase D: attention per head (K/Q chunks streamed)
            with tc.tile_pool(name="kqch", bufs=3) as kqch, \
                 tc.tile_pool(name="expool", bufs=3) as expool, \
                 tc.tile_pool(name="rzp", bufs=1) as rzp, \
                 tc.tile_pool(name="sps", bufs=2, space="PSUM") as sps, \
                 tc.tile_pool(name="ops", bufs=2, space="PSUM") as ops:
                kch = qch = None
                for h in range(NH):
                    hcc, hr = h // 2, (h % 2) * HD
                    if h % 2 == 0:
                        kch = kqch.tile([P, NT], F32R, tag="kch")
                        nc.sync.dma_start(
                            kch[:], ap2(k_drams[hcc].tensor, 0,
                                        [[NT, P], [1, NT]]))
                        qch = kqch.tile([P, NQ], F32R, tag="qch")
                        nc.sync.dma_start(
                            qch[:], ap2(q_drams[hcc].tensor, 0,
                                        [[NQ, P], [1, NQ]]))
                    po = ops.tile([HD + 1, NQ], F32, tag="o")
                    for kt in range(KT):
                        pss = sps.tile([P, NQ], F32, tag="s")
                        for qs in range(NQ // NSL):
                            nc.tensor.matmul(
                                pss[:, qs * NSL:(qs + 1) * NSL],
                                kch[hr:hr + HD, kt * P:(kt + 1) * P],
                                qch[hr:hr + HD, qs * NSL:(qs + 1) * NSL],
                                start=True, stop=True)
                        ex = expool.tile([P, NQ], F32R, tag="ex")
                        nc.scalar.activation(ex[:], pss[:], AF.Exp, scale=0.125)
                        for qs in range(NQ // NSL):
                            nc.tensor.matmul(
                                po[:, qs * NSL:(qs + 1) * NSL],
                                V_aug[:, kt, h * HC:(h + 1) * HC],
                                ex[:, qs * NSL:(qs + 1) * NSL],
                                start=(kt == 0), stop=(kt == KT - 1))
                    rz = rzp.tile([1, NQ], F32, tag="rz")
                    nc.vector.reciprocal(rz[:], po[HD:HD + 1, :])
                    rzb = rzp.tile([HD, NQ], F32, tag="rzb")
                    nc.gpsimd.partition_broadcast(rzb[:], rz[:])
                    nc.vector.tensor_tensor(oT[hr:hr + HD, hcc, :],
                                            po[0:HD, :], rzb[:], op=ALU.mult)

        # ============ Phase E: out-proj (gate1 folded into Wo, token-major
        # output) + residual + LN2
        res2 = ctx.enter_context(tc.tile_pool(name="res2", bufs=1, side="right"))
        h2T = res2.tile([P, DC, NQ], F32R)
        x2 = res2.tile([P, NQ // P, D], F32)
        with tc.tile_pool(name="mod2", bufs=1) as mod2, \
             tc.tile_pool(name="wo", bufs=1) as wop, \
             tc.tile_pool(name="ln2", bufs=3) as ln2p, \
             tc.tile_pool(name="aops", bufs=2, space="PSUM") as aops, \
             tc.tile_pool(name="tps2", bufs=4, space="PSUM") as tps2:
            g1_bc = load_bc(mod2, "g1_bc", adh, 2 * D)
            s2_pp = load_pp(mod2, "s2_pp", 4 * D, plus1=True)
            sh2_pp = load_pp(mod2, "sh2_pp", 3 * D)
            # cst_bc = g1*bo + g2*b2 (broadcast row; pre-folded into x2)
            cst_bc = mod2.tile([P, D], F32, name="cst_bc")
            tmp1 = ln2p.tile([P, D], F32, tag="tmp1", bufs=1)
            tmp2 = ln2p.tile([P, D], F32, tag="tmp2", bufs=1)
            nc.sync.dma_start(tmp1[:], ap2(bo, 0, [[0, P], [1, D]]))
            nc.vector.tensor_tensor(cst_bc[:], g1_bc[:], tmp1[:], op=ALU.mult)
            nc.sync.dma_start(tmp1[:], ap2(b2, 0, [[0, P], [1, D]]))
            nc.sync.dma_start(tmp2[:], ap2(adh, 5 * D, [[0, P], [1, D]]))
            nc.vector.tensor_tensor(tmp1[:], tmp1[:], tmp2[:], op=ALU.mult)
            nc.vector.tensor_tensor(cst_bc[:], cst_bc[:], tmp1[:], op=ALU.add)
            # Wo' rhs tiles (g1 pre-scaled), all resident, read once
            wot = {}
            for nh in range(D // 512):
                for oc in range(DC):
                    wt = wop.tile([P, 512], F32R, tag=f"w{nh}_{oc}")
                    eng = nc.scalar if oc % 2 == 0 else nc.sync
                    eng.dma_start(
                        wt[:], ap2(Wo, oc * P * D + nh * 512,
                                   [[D, P], [1, 512]]).bitcast(F32R))
                    nc.vector.tensor_tensor(
                        wt[:], wt[:], g1_bc[:, nh * 512:(nh + 1) * 512],
                        op=ALU.mult)
                    wot[(nh, oc)] = wt
            for t in range(NQ // P):
                xt = ln2p.tile([P, D], F32, tag="x")
                nc.sync.dma_start(xt[:], ap2(xb, t * P * D, [[D, P], [1, D]]))
                nc.vector.tensor_tensor(xt[:], xt[:], cst_bc[:], op=ALU.add)
                for nh in range(D // 512):
                    ps = aops.tile([P, 512], F32, tag="ps")
                    for oc in range(DC):
                        nc.tensor.matmul(ps[:], oT[:, oc, t * P:(t + 1) * P],
                                         wot[(nh, oc)][:],
                                         start=(oc == 0), stop=(oc == DC - 1))
                    sl = slice(nh * 512, (nh + 1) * 512)
                    nc.vector.tensor_tensor(x2[:, t, sl], xt[:, sl], ps[:],
                                            op=ALU.add)
                xn2 = ln_normalize(ln2p, x2[:, t, :])
                for dc in range(DC):
                    pt = tps2.tile([P, P], F32, tag="t")
                    nc.tensor.transpose(pt[:], xn2[:, dc * P:(dc + 1) * P],
                                        ident[:])
                    if dc % 2 == 0:
                        nc.vector.tensor_scalar(
                            h2T[:, dc, t * P:(t + 1) * P], pt[:],
                            scalar1=s2_pp[:, dc:dc + 1],
                            scalar2=sh2_pp[:, dc:dc + 1],
                            op0=ALU.mult, op1=ALU.add)
                    else:
                        nc.scalar.activation(
                            h2T[:, dc, t * P:(t + 1) * P], pt[:],
                            AF.Identity,
                            scale=s2_pp[:, dc:dc + 1],
                            bias=sh2_pp[:, dc:dc + 1])

        # ============ Phase F: FFN
        ff_acc = ores.tile([P, DC, NQ], F32, tag="big3")
        with tc.tile_pool(name="wff", bufs=2) as wff, \
             tc.tile_pool(name="gt", bufs=2) as gtp, \
             tc.tile_pool(name="ps2p", bufs=2, space="PSUM") as ps2p, \
             tc.tile_pool(name="gps", bufs=2, space="PSUM") as gps:
            for fb in range(FB):
                w1t, w2t = [], []
                for kc in range(DC):
                    wt = wff.tile([P, FCB * P], F32R, tag=f"w1_{kc}")
                    eng1 = nc.scalar if kc % 2 == 0 else nc.sync
                    eng1.dma_start(
                        wt[:], ap2(W1, kc * P * DFF + fb * FCB * P,
                                   [[DFF, P], [1, FCB * P]]).bitcast(F32R))
                    w1t.append(wt)
                for fc in range(FCB):
                    f = fb * FCB + fc
                    wt = wff.tile([P, D], F32R, tag=f"w2_{fc}")
                    eng2 = nc.sync if fc % 2 == 0 else nc.scalar
                    eng2.dma_start(
                        wt[:], ap2(W2, f * P * D, [[D, P], [1, D]]).bitcast(F32R))
                    w2t.append(wt)
                for qs in range(QS):
                    gts = []
                    for fc in range(FCB):
                        f = fb * FCB + fc
                        psg = gps.tile([P, NQQ], F32, tag="g")
                        for kc in range(DC):
                            nc.tensor.matmul(
                                psg[:], w1t[kc][:, fc * P:(fc + 1) * P],
                                h2T[:, kc, qs * NQQ:(qs + 1) * NQQ],
                                start=(kc == 0), stop=(kc == DC - 1))
                        g_t = gtp.tile([P, NQQ], F32R, tag=f"g{fc}")
                        if GELU_FUNC == "sigmoid_approx":
                            # CoreSim has no Gelu; x*sigmoid(1.702x) stand-in
                            xb1 = gtp.tile([P, NQQ], F32, tag="xb1")
                            nc.vector.tensor_scalar(xb1[:], psg[:],
                                                    scalar1=b1_pp[:, f:f + 1],
                                                    scalar2=None, op0=ALU.add)
                            sg = gtp.tile([P, NQQ], F32, tag="sg")
                            nc.scalar.activation(sg[:], xb1[:], AF.Sigmoid,
                                                 scale=1.702)
                            nc.vector.tensor_tensor(g_t[:], xb1[:], sg[:],
                                                    op=ALU.mult)
                        else:
                            nc.scalar.activation(g_t[:], psg[:], GELU_FUNC,
                                                 bias=b1_pp[:, f:f + 1])
                        gts.append(g_t)
                    for mc in range(DC):
                        ps2 = ps2p.tile([P, NQQ], F32, tag="ps2")
                        for fc in range(FCB):
                            nc.tensor.matmul(ps2[:], w2t[fc][:, mc * P:(mc + 1) * P],
                                             gts[fc][:],
                                             start=(fc == 0), stop=(fc == FCB - 1))
                        if fb == 0:
                            nc.scalar.copy(
                                ff_acc[:, mc, qs * NQQ:(qs + 1) * NQQ], ps2[:])
                        else:
                            nc.vector.tensor_tensor(
                                ff_acc[:, mc, qs * NQQ:(qs + 1) * NQQ],
                                ff_acc[:, mc, qs * NQQ:(qs + 1) * NQQ],
                                ps2[:], op=ALU.add)

        # ============ Phase G: out = x2 + gate2*ff (g2*b2 pre-folded in E)
        with tc.tile_pool(name="modg", bufs=1) as modg, \
             tc.tile_pool(name="fin", bufs=2) as finp, \
             tc.tile_pool(name="tps3", bufs=4, space="PSUM") as tps3:
            g2_bc = load_bc(modg, "g2_bc", adh, 5 * D)
            for t in range(NQ // P):
                ff_t = finp.tile([P, D], F32, tag="ff")
                for mc in range(DC):
                    pt = tps3.tile([P, P], F32, tag="t")
                    nc.tensor.transpose(pt[:], ff_acc[:, mc, t * P:(t + 1) * P],
                                        ident[:])
                    if mc % 2 == 0:
                        nc.vector.tensor_copy(ff_t[:, mc * P:(mc + 1) * P], pt[:])
                    else:
                        nc.scalar.copy(ff_t[:, mc * P:(mc + 1) * P], pt[:])
                o_t = finp.tile([P, D], F32, tag="o")
                nc.vector.tensor_tensor(o_t[:], ff_t[:], g2_bc[:], op=ALU.mult)
                nc.vector.tensor_tensor(o_t[:], o_t[:], x2[:, t, :], op=ALU.add)
                nc.sync.dma_start(ap2(out, t * P * D, [[D, P], [1, D]]), o_t[:])

    return {"ada": ada_dram.tensor.name}


_COMPILED = None


def _get_compiled():
    global _COMPILED
    if _COMPILED is None:
        from concourse import bacc
        nc = bacc.Bacc("TRN2", target_bir_lowering=False, debug=False,
                       enable_partition_id=False)
        build_dit(nc)
        nc.compile()
        _COMPILED = nc
    return _COMPILED


def kernel(x, c, W_ada, b_ada, Wq, bq, Wk, bk, Wv, bv, Wo, bo, W1, b1, W2, b2):
    from concourse import bass_utils
    nc = _get_compiled()
    B, N, D = x.shape
    assert (B, N, D) == (4, 2048, 1024)
    x = np.ascontiguousarray(np.asarray(x, dtype=np.float32))
    shared = {
        "W_ada": np.asarray(W_ada, np.float32),
        "b_ada": np.asarray(b_ada, np.float32).reshape(1, -1),
        "Wq": np.asarray(Wq, np.float32), "bq": np.asarray(bq, np.float32).reshape(1, -1),
        "Wk": np.asarray(Wk, np.float32), "bk": np.asarray(bk, np.float32).reshape(1, -1),
        "Wv": np.asarray(Wv, np.float32), "bv": np.asarray(bv, np.float32).reshape(1, -1),
        "Wo": np.asarray(Wo, np.float32), "bo": np.asarray(bo, np.float32).reshape(1, -1),
        "W1": np.asarray(W1, np.float32), "b1": np.asarray(b1, np.float32).reshape(1, -1),
        "W2": np.asarray(W2, np.float32), "b2": np.asarray(b2, np.float32).reshape(1, -1),
    }
    in_maps = []
    for core in range(8):
        b, s = core // 2, core % 2
        xb_ = np.roll(x[b], -1024 * s, axis=0) if s else x[b]
        m = dict(shared)
        m["xb"] = np.ascontiguousarray(xb_)
        m["cb"] = np.ascontiguousarray(np.asarray(c, np.float32)[b:b + 1])
        in_maps.append(m)

    last_err = None
    for _attempt in range(3):
        try:
            res = bass_utils.run_bass_kernel_spmd(nc, in_maps, core_ids=list(range(8)))
            break
        except Exception as e:  # transient NRT device errors; retry
            last_err = e
    else:
        raise last_err

    out = np.empty((4, 2048, 1024), np.float32)
    for core in range(8):
        b, s = core // 2, core % 2
        out[b, s * 1024:(s + 1) * 1024, :] = res.results[core]["out"]
    return out

